# revision 55
# baseline (speedup 1.0000x reference)
"""Trainium2 Bass kernel for the class-balanced supervised-contrastive loss.

Math (reference semantics, shift-invariant form with constant shift 10):
  l_ij = (f_i . g_j) / T,  T = 0.1, g = [features; centers; features_ood]
  E_ij = exp(l_ij - 10)
  S_i  = sum_{j != i} E_ij / (w_j - eq_ij)        (w_j = class count, eq = label match)
  P_i  = sum_{j != i} eq_ij (l_ij - 10)
  loss = -mean_i( P_i / K_i - log S_i ),  K_i = batch count of class t_i

Device strategy (per core, rows globally sorted by class and sharded 512/core):
  All dot products run as single-term fp8e4m3 DoubleRow matmuls (K=256 per
  instruction, 0.5 cycles/col: 4x faster than bf16).  f and g are scaled by
  16 before quantization so entries sit in e4m3's normal range; the PSUM
  then carries 256*r and every consumer rescales (ACT scale=10/256, host
  divides S3/diag by 256).  fp8 quantization noise is ~1e-3 (sigma) on r:
  mean-zero for the big sums, and the dominant self column cancels exactly
  because diag is read back from the very same PSUM value that produced the
  denominator's self term.  Host emulation: 5e-5 end-to-end rel err.

  Columns are permuted per core to [own 512 | matched | rest | ood | pad] so
  all label matches live in the first 768 columns ("eq" region):
    eq:    psum = ones2^T@(256*bias hi;lo)[bf16] + fp8 DR;  ACT exp fp32 with
           accum -> A_eq; DVE is_eq stts on two 256-col windows -> S2, S3;
           an identity-masked stt on the psum diagonal block -> diag.
    cheap: 2 warm-up pairs + 4 triads of psum.  Most tiles: ACT exp -> fp16
           E, DVE stt E * (2^14*cw)[fp16] with accum -> A partial.  The last
           triad folds 25.6*ln(2^14*cw) into a bf16 bias matmul so the ACT
           accum produces the partial directly — balancing ACT (~37us) and
           DVE (~36us), the two near-equal bottleneck engines, and leaving
           the kernel tail DVE-free.
  The Exp activation table load hides behind the DMA head via a dummy
  activation; the first g chunks ride the Pool/ACT DMA queues in parallel
  with the SP queue.  All partials accumulate straight into the output tile;
  the final cross-partial sums are host-side O(B) math.  PSUM matmul
  accumulation regions are kept 512-col bank-aligned throughout (misaligned
  regions corrupt accumulation nondeterministically).
"""

import ml_dtypes
import numpy as np

import concourse.bass as bass
import concourse.mybir as mybir
import concourse.tile as tile
from concourse.bass_utils import run_bass_kernel_spmd

NCORES = 8
C, TEMP = 1000, 0.1
B, BO, D = 4096, 4096, 512
N = B + C + BO              # 9192
NPAD = 9216                 # 18 * 512
PAD = NPAD - N
NCH = NPAD // 512           # 18 column chunks
RPC = B // NCORES           # 512 rows per core
MT = RPC // 128             # 4 row tiles per core

FSCALE = 16.0               # fp8 quantization scale for f and g
PSC = FSCALE * FSCALE       # psum carries PSC * r
ACT_SCALE = 10.0 / PSC      # exp(10*r) = exp(ACT_SCALE * psum)
CW_SCALE = 2.0 ** 14        # keeps cw out of fp16 subnormal range
# cheap chunk grouping: two [128,1024] warm-up pairs (the second also absorbs
# the 256 columns left over from the eq region), then four [128,1536] triads;
# the last triad's weighting runs via bias-fold + ACT accum so the kernel
# tail has no DVE work.
TRIAD_CH = (6, 9, 12, 15)   # first chunk of each triad
ACT_GROUP = 15              # bias-fold triad (includes the pad columns)
EQN = 768                   # eq-region width: own 512 + (matched <= W1)
W1 = 192                    # matched-window width for the S2/S3 stts
NS = 12                     # out cols per m: a_p0,a_p1,a_t0..3,a_eq,s2a,s2b,s3a,s3b,diag

F32 = mybir.dt.float32
F16 = mybir.dt.float16
BF16 = mybir.dt.bfloat16
FP8 = mybir.dt.float8e4
ALU = mybir.AluOpType
AF = mybir.ActivationFunctionType
DR = mybir.MatmulPerfMode.DoubleRow
BFNP = ml_dtypes.bfloat16
E4NP = ml_dtypes.float8_e4m3

# This walrus build accepts only one sync-wait command per engine instruction.
# Move surplus waits onto standalone EventSemaphore instructions just before
# the affected instruction (same engine, so blocking semantics are identical).
_SPLIT_SKIP = ("InstEventSemaphore",)


def _split_multi_waits(nc):
    n = 0
    for f in nc.m.functions:
        for bb in f.blocks:
            new = []
            for ins in bb.instructions:
                si = ins.sync_info
                if (
                    si is not None
                    and si.on_wait
                    and len(si.on_wait) > 1
                    and type(ins).__name__ not in _SPLIT_SKIP
                ):
                    waits = list(si.on_wait)
                    for w in waits[:-1]:
                        es = mybir.InstEventSemaphore(
                            name=f"wsplit_{n}",
                            engine=ins.engine,
                            sync_info=mybir.SyncInfo(on_wait=[w], on_update=[]),
                        )
                        n += 1
                        new.append(es)
                    ins.sync_info = mybir.SyncInfo(
                        on_wait=[waits[-1]], on_update=list(si.on_update)
                    )
                new.append(ins)
            bb.instructions = new
    return n


def _build_nc(woff=None):
    """woff: per-m 256-col window offsets for chunk-0 stts (None = full 512)."""
    nc = bass.Bass()
    g8 = nc.declare_dram_parameter("g8", [128, NCH, 4, 512], FP8, isOutput=False)
    f8 = nc.declare_dram_parameter("f8", [128, 4, 512], FP8, isOutput=False)
    # row 0 = (ones128, 256*bias_hi for eq cols), row 1 = lo parts
    cst = nc.declare_dram_parameter("cst", [2, 128 + EQN], BF16, isOutput=False)
    # 25.6*ln(2^14*cw) hi/lo rows for the bias-fold triad's columns
    cwb = nc.declare_dram_parameter("cwb", [2, 1536], BF16, isOutput=False)
    cw16 = nc.declare_dram_parameter("cw16", [128, NPAD - EQN], F16, isOutput=False)
    ta = nc.declare_dram_parameter("ta", [128, EQN], F16, isOutput=False)
    tvec = nc.declare_dram_parameter("tvec", [128, MT], F32, isOutput=False)
    ident = nc.declare_dram_parameter("ident", [128, 128], F32, isOutput=False)
    out = nc.declare_dram_parameter("out", [128, NS * MT], F32, isOutput=True)

    with tile.TileContext(nc) as tc:
        with (
            tc.tile_pool(name="const", bufs=1) as const,
            tc.tile_pool(name="stats", bufs=1) as stats,
            tc.tile_pool(name="gt", bufs=3) as gtp,
            tc.tile_pool(name="e1", bufs=5) as e1p,
            tc.tile_pool(name="scr", bufs=3) as scr,
            tc.tile_pool(name="psum", bufs=2, space="PSUM") as psp,
            tc.tile_pool(name="pseq", bufs=1, space="PSUM") as pseqp,
        ):
            # dummy exp so the ACT table load hides inside the DMA head
            dum = const.tile([128, 8], F32)
            nc.vector.memset(dum[:], 0.0)
            nc.scalar.activation(dum[:], dum[:], AF.Exp, scale=1.0)

            f8s = const.tile([128, 4, 512], FP8)
            nc.sync.dma_start(out=f8s[:], in_=f8[:])
            cwb_sb = const.tile([2, 1536], BF16)
            cst_sb = const.tile([2, 128 + EQN], BF16)
            ta_sb = const.tile([128, EQN], F16)
            tvec_sb = const.tile([128, MT], F32)
            ident_sb = const.tile([128, 128], F32)
            cw_sb = const.tile([128, NPAD - EQN], F16)
            ones_sb = cst_sb[:, 0:128]
            brow_sb = cst_sb[:, 128:128 + EQN]

            outsb = stats.tile([128, NS * MT], F32)

            def dr_group(ps_seg, gt, ci, off, n, m, with_bias=None):
                """Accumulate cols [off, off+n) of tile-chunk ci into ps_seg:
                optional bf16 bias matmul + 2 fp8 DoubleRow matmuls."""
                if with_bias is not None:
                    nc.tensor.matmul(ps_seg, ones_sb, with_bias, start=True, stop=False)
                for kp in range(2):
                    nc.tensor.matmul(
                        ps_seg,
                        f8s[:, 2 * kp:2 * kp + 2, 128 * m:128 * (m + 1)],
                        gt[:, ci, 2 * kp:2 * kp + 2, off:off + n],
                        start=(with_bias is None and kp == 0),
                        stop=(kp == 1),
                        perf_mode=DR,
                    )

            def emit_eq(geq, ms):
                # Eq region [own 512 | matched+rest 256]: one psum tile per m;
                # bias included so ACT accum gives A_eq and the psum carries
                # 256*(r + bias).
                for m in ms:
                    ps = pseqp.tile([128, EQN], F32, name="pse", tag="pse")
                    dr_group(ps[:, 0:512], geq, 0, 0, 512, m,
                             with_bias=brow_sb[:, 0:512])
                    dr_group(ps[:, 512:EQN], geq, 1, 0, EQN - 512, m,
                             with_bias=brow_sb[:, 512:EQN])
                    e1 = e1p.tile([128, EQN], F32, name="e1", tag="e1")
                    nc.scalar.activation(
                        e1[:], ps[:], AF.Exp, scale=ACT_SCALE,
                        accum_out=outsb[:, NS * m + 6:NS * m + 7],
                    )
                    # stt windows: all matches (incl the diagonal) live here
                    wins = [(woff[m], 256) if woff is not None else (0, 512),
                            (512, W1)]
                    for wi, (off, wn) in enumerate(wins):
                        ws_ = slice(off, off + wn)
                        sc2 = scr.tile([128, 512], F32, name="sc2", tag="scr2")
                        nc.vector.scalar_tensor_tensor(
                            out=sc2[:, 0:wn],
                            in0=ta_sb[:, ws_],
                            scalar=tvec_sb[:, m:m + 1],
                            in1=e1[:, ws_],
                            op0=ALU.is_equal,
                            op1=ALU.mult,
                            accum_out=outsb[:, NS * m + 7 + wi:NS * m + 8 + wi],
                        )
                        sc3 = scr.tile([128, 512], F32, name="sc3", tag="scr3")
                        nc.vector.scalar_tensor_tensor(
                            out=sc3[:, 0:wn],
                            in0=ta_sb[:, ws_],
                            scalar=tvec_sb[:, m:m + 1],
                            in1=ps[:, ws_],
                            op0=ALU.is_equal,
                            op1=ALU.mult,
                            accum_out=outsb[:, NS * m + 9 + wi:NS * m + 10 + wi],
                        )
                    if wi == 1:
                        # local row p's own column is chunk-0 column 128m+p:
                        # the psum diagonal of this [128,128] sub-block is the
                        # self dot product (plus bias) bit-exactly as it
                        # entered A_eq.
                        sd = scr.tile([128, 128], F32, name="sd", tag="scrd")
                        nc.vector.scalar_tensor_tensor(
                            out=sd[:],
                            in0=ident_sb[:],
                            scalar=1.0,
                            in1=ps[:, 128 * m:128 * (m + 1)],
                            op0=ALU.mult,
                            op1=ALU.mult,
                            accum_out=outsb[:, NS * m + 11:NS * m + 12],
                        )

            def cheap_tiles(segs, cw_off, slot, act_ms, gts, ms):
                """One cheap group (segs = [(tile, ci, off, n), ...]) for row
                tiles ms; accum lands in outsb column `slot` of each m."""
                wid = sum(n for _, _, _, n in segs)
                for m in ms:
                    use_act = m in act_ms
                    ps = psp.tile([128, wid], F32, name="psc", tag="ps")
                    po = 0
                    for gt, ci, off, n in segs:
                        bias = (cwb_sb[:, po:po + n] if use_act else None)
                        dr_group(ps[:, po:po + n], gt, ci, off, n, m,
                                 with_bias=bias)
                        po += n
                    e16 = e1p.tile([128, wid], F16, name="e16", tag="e16")
                    acc = outsb[:, NS * m + slot:NS * m + slot + 1]
                    if use_act:
                        nc.scalar.activation(e16[:], ps[:], AF.Exp,
                                             scale=ACT_SCALE, accum_out=acc)
                    else:
                        nc.scalar.activation(e16[:], ps[:], AF.Exp, scale=ACT_SCALE)
                        sc = scr.tile([128, 1536], F16, name="scw", tag="scrw")
                        nc.vector.scalar_tensor_tensor(
                            out=sc[:, 0:wid],
                            in0=e16[:],
                            scalar=1.0,
                            in1=cw_sb[:, cw_off:cw_off + wid],
                            op0=ALU.mult,
                            op1=ALU.mult,
                            accum_out=acc,
                        )

            def load_g(ch0, nch, halves, engines=None):
                gt = gtp.tile([128, nch, 4, 512], FP8, name="gtc", tag="gtc")
                step = nch // halves
                for h in range(halves):
                    c0 = ch0 + step * h
                    eng = (engines[h] if engines else nc.sync)
                    eng.dma_start(out=gt[:, step * h:step * h + step],
                                  in_=g8[:, c0:c0 + step])
                return gt

            def load_cw(cw_off, wid):
                nc.sync.dma_start(out=cw_sb[:, cw_off:cw_off + wid],
                                  in_=cw16[:, cw_off:cw_off + wid])

            # DMA order is queue order: the first pair's chunks ride the
            # Pool/ACT DMA queues so they land in parallel with f8 on SP,
            # and the eq-phase constants slot in behind them.
            gp0 = load_g(2, 2, 2, engines=(nc.scalar, nc.gpsimd))
            geq = load_g(0, 2, 1, engines=(nc.gpsimd,))
            nc.scalar.dma_start(out=cst_sb[:], in_=cst[:])
            load_cw(0, 1024)
            gp1 = load_g(4, 2, 1)
            nc.sync.dma_start(out=ta_sb[:], in_=ta[:])
            nc.sync.dma_start(out=tvec_sb[:], in_=tvec[:])
            nc.sync.dma_start(out=ident_sb[:], in_=ident[:])
            load_cw(1024, 1280)
            nc.sync.dma_start(out=cwb_sb[:], in_=cwb[:])

            # interleave the long-psum-lifetime eq tiles with the warm-up
            # pairs so the psum slots stay productive.
            # psum matmul regions must be bank-aligned: keep the 256-wide
            # leftover seg at a 512-col boundary
            p0segs = [(gp0, 0, 0, 512), (gp0, 1, 0, 512)]
            p1segs = [(gp1, 0, 0, 512), (gp1, 1, 0, 512), (geq, 1, 256, 256)]
            cheap_tiles(p0segs, 0, 0, (), None, ms=(0, 1))
            emit_eq(geq, ms=(0,))
            cheap_tiles(p0segs, 0, 0, (), None, ms=(2, 3))
            emit_eq(geq, ms=(1,))
            cheap_tiles(p1segs, 1024, 1, (), None, ms=(0, 1))
            emit_eq(geq, ms=(2,))
            cheap_tiles(p1segs, 1024, 1, (), None, ms=(2, 3))
            emit_eq(geq, ms=(3,))
            for qi, ch0 in enumerate(TRIAD_CH):
                act_ms = (0, 1, 2, 3) if ch0 == ACT_GROUP else ()
                gt = load_g(ch0, 3, 1, engines=(nc.gpsimd,))
                cw_off = 2304 + 1536 * qi
                load_cw(cw_off, 1536)
                segs = [(gt, ci, 0, 512) for ci in range(3)]
                cheap_tiles(segs, cw_off, 2 + qi, act_ms, None, ms=range(MT))

            nc.sync.dma_start(out=out[:], in_=outsb[:])
    _split_multi_waits(nc)
    return nc


_nc_by_cfg = {}


def _get_nc(woff):
    if woff not in _nc_by_cfg:
        _nc_by_cfg[woff] = _build_nc(woff)
    return _nc_by_cfg[woff]


def _tile_cols(x):
    """[ncols, 512] -> [128, ncols/512, 4, 512] in the DoubleRow SBUF layout:
    [p, ch, k, j] = x[512*ch + j, 128*k + p]."""
    nch = x.shape[0] // 512
    xt = np.ascontiguousarray(x.T)                     # [512(d), ncols]
    return np.ascontiguousarray(
        xt.reshape(4, 128, nch, 512).transpose(1, 2, 0, 3)
    )


def _prepare(centers1, features, targets, features_ood, pseudo_target_ood):
    """Host-side O(N log N) prep: sort rows by class, shard contiguously,
    per-core column permutation [own | matched | rest | ood | pad], fp8
    quantization, and the small bf16/fp16 side tensors."""
    centers1 = np.asarray(centers1, np.float32)
    features = np.asarray(features, np.float32)
    features_ood = np.asarray(features_ood, np.float32)
    targets = np.asarray(targets).astype(np.int64)
    pseudo = np.asarray(pseudo_target_ood).astype(np.int64)

    tac = np.concatenate([targets, np.arange(C), pseudo])
    w_full = np.bincount(tac, minlength=C).astype(np.float64)

    # class-id label per g row (incl. centers/ood) and per-row bias
    lab = np.concatenate([targets, np.arange(C), np.full(BO, C, np.int64),
                          np.full(PAD, -1, np.int64)])
    bias1 = -(np.log(w_full[tac]) + 10.0) / 10.0
    bias_s = np.zeros(NPAD, np.float64)
    bias_s[:N] = bias1 * PSC
    b_h = bias_s.astype(BFNP)
    b_l = (bias_s - b_h.astype(np.float64)).astype(BFNP)

    g = np.concatenate(
        [features, centers1, features_ood, np.zeros((PAD, D), np.float32)], axis=0
    )
    g8 = (g * FSCALE).astype(E4NP)

    cw_row = np.zeros(NPAD, np.float64)
    cw_row[:N] = np.exp(-10.0) / w_full[tac] * CW_SCALE
    # bias-fold form for ACT_TILES: exp(ACT_SCALE*psum + ln cw') needs
    # 25.6*ln(cw') in the psum; pad columns get a large negative bias.
    cwb_full = np.full(NPAD, -60.0 * PSC / 10.0, np.float64)
    cwb_full[:N] = np.log(cw_row[:N]) * PSC / 10.0

    row_perm = np.argsort(targets, kind="stable")
    t_sorted = targets[row_perm]

    ident = np.eye(128, dtype=np.float32)
    ones2 = np.ones((2, 128), BFNP)

    # per-core column permutations
    perms = []
    woff_ok = True
    WOFF = (0, 64, 192, 256)
    all_batch = np.arange(B)
    for c in range(NCORES):
        own = row_perm[RPC * c:RPC * (c + 1)]              # sorted by class
        tc_ = t_sorted[RPC * c:RPC * (c + 1)]
        tset = np.zeros(C + 1, bool)
        tset[tc_] = True
        in_own = np.zeros(B, bool)
        in_own[own] = True
        match_b = all_batch[tset[targets] & ~in_own]       # other cores' rows, own classes
        match_c = B + np.flatnonzero(tset[:C])             # centers of own classes
        matched = np.concatenate([match_b, match_c])
        assert len(matched) <= W1, len(matched)
        rest_mask = np.ones(B + C, bool)
        rest_mask[own] = False
        rest_mask[matched] = False
        rest = np.flatnonzero(rest_mask)
        perm = np.concatenate(
            [own, matched, rest,
             np.arange(B + C, N),                          # ood
             np.arange(N, NPAD)]                           # pad
        )
        assert perm.shape == (NPAD,)
        perms.append(perm)
        # chunk-0 window check: row-tile m only matches own columns whose
        # classes occur in its rows — a narrow band around 128*m.
        for m in range(MT):
            lo = np.searchsorted(tc_, tc_[128 * m], side="left")
            hi = np.searchsorted(tc_, tc_[128 * m + 127], side="right")
            if not (WOFF[m] <= lo and hi <= WOFF[m] + 256):
                woff_ok = False

    woff = WOFF if woff_ok else None

    in_maps = []
    for c in range(NCORES):
        perm = perms[c]
        g8_c = _tile_cols(g8[perm])                        # [128, 18, 4, 512]
        f8_c = np.ascontiguousarray(g8_c[:, 0])            # own rows = chunk 0
        bh_p, bl_p = b_h[perm[:EQN]], b_l[perm[:EQN]]
        cst_c = np.ascontiguousarray(
            np.concatenate([ones2, np.stack([bh_p, bl_p])], axis=1).astype(BFNP)
        )
        cwb_q = cwb_full[perm[ACT_GROUP * 512:(ACT_GROUP + 3) * 512]]
        cwb_h = cwb_q.astype(BFNP)
        cwb_l = (cwb_q - cwb_h.astype(np.float64)).astype(BFNP)
        cwb_c = np.ascontiguousarray(np.stack([cwb_h, cwb_l]))
        # cheap columns in device group order: pair0 (ch2,3), pair1 (ch4,5 +
        # the 256 eq leftovers), then the four triads
        cheap_cols = np.concatenate(
            [perm[1024:2048], perm[2048:3072], perm[768:1024], perm[3072:]]
        )
        cw_p = cw_row[cheap_cols].astype(np.float16)
        cw_bc = np.ascontiguousarray(np.broadcast_to(cw_p, (128, NPAD - EQN)))
        ta_p = lab[perm[:EQN]].astype(np.float16)
        ta_bc = np.ascontiguousarray(np.broadcast_to(ta_p, (128, EQN)))
        tvec_c = np.ascontiguousarray(
            t_sorted[RPC * c:RPC * (c + 1)].reshape(MT, 128).T.astype(np.float32)
        )
        in_maps.append(
            {
                "g8": g8_c,
                "f8": f8_c,
                "cst": cst_c,
                "cwb": cwb_c,
                "cw16": cw_bc,
                "ta": ta_bc,
                "tvec": tvec_c,
                "ident": ident,
            }
        )

    # effective per-class bias as the device psum sees it (fp32 add of pair)
    cls_bias = -(np.log(w_full) + 10.0) * PSC / 10.0
    cb_h = cls_bias.astype(BFNP)
    cb_l = (cls_bias - cb_h.astype(np.float64)).astype(BFNP)
    bias_eff_cls = (cb_h.astype(np.float32)
                    + cb_l.astype(np.float32)).astype(np.float64) / PSC

    host = {"t_sorted": t_sorted, "w_full": w_full, "bias_eff_cls": bias_eff_cls,
            "woff": woff}
    return in_maps, host


def _combine(results, host):
    t_sorted = host["t_sorted"]
    w_full = host["w_full"]
    cnt_batch = np.bincount(t_sorted, minlength=C).astype(np.float64)

    A = np.empty(B)
    S2 = np.empty(B)
    S3 = np.empty(B)
    diag = np.empty(B)
    for c in range(NCORES):
        o = np.asarray(results[c]["out"], np.float64)  # [128, NS*MT]
        for m in range(MT):
            rs = slice(RPC * c + 128 * m, RPC * c + 128 * (m + 1))
            A[rs] = o[:, NS * m:NS * m + 6].sum(1) / CW_SCALE + o[:, NS * m + 6]
            S2[rs] = o[:, NS * m + 7] + o[:, NS * m + 8]
            S3[rs] = (o[:, NS * m + 9] + o[:, NS * m + 10]) / PSC
            diag[rs] = o[:, NS * m + 11] / PSC

    ws = w_full[t_sorted]
    K = cnt_batch[t_sorted]
    ds_ = 1.0 / (ws - 1.0) - 1.0 / ws
    b1s = host["bias_eff_cls"][t_sorted]
    e1s = np.exp(10.0 * diag)
    S = A - e1s + ds_ * ws * (S2 - e1s)
    P = 10.0 * (S3 - K * b1s - diag) - 10.0 * K
    val = P / K - np.log(S)
    return np.float32(-val.mean())


def _run(inputs, trace=False, **kw):
    in_maps, host = _prepare(**inputs)
    nc = _get_nc(host["woff"])
    res = run_bass_kernel_spmd(nc, in_maps, list(range(NCORES)), trace=trace, **kw)
    loss = _combine(res.results, host)
    return loss, res


def kernel(**inputs):
    loss, _ = _run(inputs)
    return loss
